# revision 1
# baseline (speedup 1.0000x reference)
"""Chamfer distance kernel for Trainium2 (Bass/Tile), 8-core SPMD.

Problem: recon/target [64, 4, 2048] f32, mask [64, 2048] i32 ->
scalar mean chamfer loss (squared distances, masked min both directions).

Strategy (data-parallel over batch, 8 samples/core):
  - For each sample the halved negated pairwise distance matrix
        V[n, m] = x_n . y_m - (xn[n] + BIGr[n])/2 - (yn[m] + BIGc[m])/2
    is produced by ONE K=16 bf16 matmul per tile using an error-free-style
    split (x = xhi + xlo in bf16; dot = xhi.yhi + xhi.ylo + xlo.yhi, the
    dropped xlo.ylo term is ~2^-18 relative).  bf16 matmuls stream at
    1 cycle/column (fp32 is 4x slower on the PE), and bf16xbf16 products
    accumulate exactly in fp32 PSUM, so this is fp32-grade accuracy at 4x
    the speed.  Norm rows are hi/lo split the same way.  Row-side vectors
    carry +BIG*(1-mask)/2 (invalid rows -> V=+BIG/2 -> relu(-2*max)=0: no
    mask multiply needed), column-side vectors carry -BIG*(1-mask)/2
    (invalid columns excluded from the max).  max_m V = -d2min/2, recovered
    exactly by relu(-2*max) in the epilogue (the clamp commutes with min).
  - Per 128-row block the PE fills PSUM [128, 2048] as two [128,1024] tiles;
    ScalarE stages the second half to SBUF; one VectorE MAX2_REDUCE custom-DVE
    op (authored here: out = max(in0,in1), accum_out = max-reduce) absorbs
    both halves at 2 elem/lane/cycle and emits the row max directly.  Both
    chamfer directions run as separate matmul orientations (x-rows / y-rows).
  - Four samples pack per 128-partition operand tensor at 32-partition slots
    (matmul lhsT base-partition constraint), with explicit tile_position.
  - Epilogue: relu(-2*max) on ScalarE, partition sum via ones-matmul, block
    sums via a 3D-AP reduce.  Output per core: sums [2, 8] + cnt [8, 1]; the
    masked means and batch mean happen on host.
"""

import sys

import numpy as np

for _p in ("/opt/trn_rl_repo",):
    if _p not in sys.path:
        sys.path.append(_p)

B, F, N = 64, 4, 2048
N_CORES = 8
SPC = B // N_CORES  # samples per core
NB = N // 128  # 128-row blocks per sample
BIGV = 1.0e30
NEG_INIT = -3.0e38

_CACHE = {}


def _register_max2_reduce():
    """Author + register a custom DVE op: out = max(in0, in1),
    accum_out = max-reduce(out) seeded from s0.  Absorbs two tiles per pass
    (one read port each) with the row-max fused — the core absorption
    primitive of this kernel."""
    from concourse import dve_ops
    from concourse.dve_spec import Spec, Src0, Src1, C0, maxx, lower, _has_src1
    from concourse.dve_uop import DveOpSpec

    NAME = "MAX2_REDUCE_ANT"
    for op in dve_ops.OPS:
        if op.name == NAME:
            return op

    def _ref_max2(in0, in1, c0, c1, c2):
        b = np.maximum(in0.astype(np.float32), in1.astype(np.float32))
        a = np.maximum(b.reshape(b.shape[0], -1).max(axis=-1, keepdims=True), c0)
        return b, a

    spec = Spec(body=maxx(Src0, Src1), accum=maxx, accum_init=C0,
                reference=_ref_max2)
    row = dve_ops._CUSTOM_DVE_ROW_BASE + len(dve_ops.OPS)
    shas = {}
    for ver in ("v3", "v4"):
        s = DveOpSpec(name=NAME, opcode=row, uops=lower(spec, ver=ver),
                      rd1_en=_has_src1(spec))
        shas[ver] = s.sha(ver)
    op = dve_ops.DveOp(NAME, spec, subdim=False, uops_sha=shas)
    dve_ops.OPS.append(op)
    dve_ops._SUB_OPCODE_FOR_NAME[NAME] = row
    dve_ops.CUSTOM_DVE_SPECS[NAME] = spec
    return op


def _build_bass():
    from contextlib import ExitStack

    import concourse.mybir as mybir
    import concourse.tile as tile
    from concourse import bacc

    max2 = _register_max2_reduce()

    f32 = mybir.dt.float32
    bf16 = mybir.dt.bfloat16
    Alu = mybir.AluOpType
    Act = mybir.ActivationFunctionType
    Axis = mybir.AxisListType

    nc = bacc.Bacc("TRN2", target_bir_lowering=False, debug=False,
                   num_devices=N_CORES)

    recon = nc.dram_tensor("recon", (SPC, F, N), f32, kind="ExternalInput").ap()
    target = nc.dram_tensor("target", (SPC, F, N), f32, kind="ExternalInput").ap()
    maskf = nc.dram_tensor("maskf", (SPC, N), f32, kind="ExternalInput").ap()
    sums_out = nc.dram_tensor("sums", (2, SPC), f32, kind="ExternalOutput").ap()
    cnt_out = nc.dram_tensor("cnt", (SPC, 1), f32, kind="ExternalOutput").ap()

    with tile.TileContext(nc) as tc, ExitStack() as ctx:
        # ---- persistent pools ----
        consts = ctx.enter_context(tc.tile_pool(name="consts", bufs=1))
        opnds = ctx.enter_context(tc.tile_pool(name="opnds", bufs=1))
        accum = ctx.enter_context(tc.tile_pool(name="accum", bufs=1))

        ones_col = consts.tile([128, 1], f32)
        nc.gpsimd.memset(ones_col, 1.0)
        ones2 = consts.tile([2, N], bf16)
        nc.gpsimd.memset(ones2, 1.0)
        # negE64 [64, 8]: -0.5 on the (4-row) block diagonal, replicated at
        # partition 0 (for x) and partition 32 (for y)
        negE = consts.tile([2 * SPC * F, SPC], f32, name="negE")
        nc.gpsimd.memset(negE, -0.5)
        for base in (0, 32):
            nc.gpsimd.affine_select(out=negE[base:base + 32, :],
                                    in_=negE[base:base + 32, :],
                                    compare_op=Alu.is_ge, fill=0.0,
                                    base=0, pattern=[[-F, SPC]],
                                    channel_multiplier=1)
            nc.gpsimd.affine_select(out=negE[base:base + 32, :],
                                    in_=negE[base:base + 32, :],
                                    compare_op=Alu.is_ge, fill=0.0,
                                    base=F - 1, pattern=[[F, SPC]],
                                    channel_multiplier=-1)

        m_sb = opnds.tile([SPC, N], f32)
        nc.sync.dma_start(out=m_sb, in_=maskf)

        # operand tensors (bf16): [orientation][group]; sample slot s lives at
        # partitions [32s, 32s+16):
        #   lhsT rows: 0-3 xhi | 4-7 xhi | 8-11 xlo | 12 rvh | 13 rvl | 14-15 1
        #   rhs  rows: 0-3 yhi | 4-7 ylo | 8-11 yhi | 12-13 1 | 14 cvh | 15 cvl
        lhsT_t = [[opnds.tile([128, N], bf16, tag=f"L{o}{g}", name=f"L{o}{g}")
                   for g in range(2)] for o in range(2)]
        rhs_t = [[opnds.tile([128, N], bf16, tag=f"R{o}{g}", name=f"R{o}{g}")
                  for g in range(2)] for o in range(2)]
        negmax = [accum.tile([128, 128], f32, tag=f"nm{o}", name=f"nm{o}")
                  for o in range(2)]

        # ---- prep: hi/lo splits, norms, masked norm vectors, assembly ----
        # prep_a holds the large f32 staging (freed before the main loop so
        # the stage pool reuses ONLY this early-released memory); prep_b holds
        # the bf16 split products consumed by the assembly DMAs.
        with tc.tile_pool(name="prep_a", bufs=1) as prep_a, \
                tc.tile_pool(name="prep_b", bufs=1) as prep_b, \
                tc.tile_pool(name="prep_ps", bufs=1, space="PSUM") as prep_ps:
            # x at partitions 0-31, y at partitions 32-63
            xy = prep_a.tile([2 * SPC * F, N], f32, tag="xy")
            nc.sync.dma_start(out=xy[:SPC * F, :],
                              in_=recon.rearrange("b f n -> (b f) n"))
            nc.sync.dma_start(out=xy[SPC * F:, :],
                              in_=target.rearrange("b f n -> (b f) n"))

            def hilo(src, tag, rows=128):
                """bf16 hi/lo split: hi = bf16(src), lo = bf16(src - hi).
                The f32 diff scratch shares one slot across all splits."""
                p = src.shape[0]
                hi = prep_b.tile([p, N], bf16, tag=f"{tag}_h", name=f"{tag}_h")
                df = prep_a.tile([128, N], f32, tag="hilo_d", name=f"{tag}_d")
                lo = prep_b.tile([p, N], bf16, tag=f"{tag}_l", name=f"{tag}_l")
                nc.scalar.copy(hi, src)
                nc.vector.tensor_sub(df[:p], src, hi)
                nc.scalar.copy(lo, df[:p])
                return hi, lo

            xyh, xyl = hilo(xy, "xy")

            sq = prep_a.tile([2 * SPC * F, N], f32, tag="sq")
            nc.scalar.square(sq[:SPC * F, :], xy[:SPC * F, :])
            nc.scalar.square(sq[SPC * F:, :], xy[SPC * F:, :])

            # -xn/2, -yn/2 via block-diagonal -(1/2) ones matmuls (K=32, M=8)
            ps_xn = prep_ps.tile([SPC, N], f32, tag="psxn")
            ps_yn = prep_ps.tile([SPC, N], f32, tag="psyn")
            for c in range(N // 512):
                sl = slice(c * 512, (c + 1) * 512)
                nc.tensor.matmul(ps_xn[:, sl], negE[0:32, :], sq[0:32, sl],
                                 start=True, stop=True, tile_position=(0, 0))
                nc.tensor.matmul(ps_yn[:, sl], negE[32:64, :], sq[32:64, sl],
                                 start=True, stop=True, tile_position=(32, 0))

            # all four masked norm vectors in one tensor (32-aligned slots):
            # rows 0-7 xr | 32-39 xc | 64-71 yr | 96-103 yc
            nf = prep_a.tile([128, N], f32, tag="nf")
            nc.gpsimd.memset(nf, 0.0)

            # BIG masks (halved): bp = +BIG*(1-m)/2, bn = -BIG*(1-m)/2
            bp = prep_a.tile([SPC, N], f32, tag="bp")
            bn = prep_a.tile([SPC, N], f32, tag="bn")
            nc.vector.tensor_scalar(out=bp, in0=m_sb, scalar1=-1.0,
                                    scalar2=-BIGV / 2, op0=Alu.add,
                                    op1=Alu.mult)
            nc.vector.tensor_scalar(out=bn, in0=m_sb, scalar1=-1.0,
                                    scalar2=BIGV / 2, op0=Alu.add,
                                    op1=Alu.mult)
            nc.vector.tensor_add(nf[0:SPC, :], ps_xn, bp)
            nc.vector.tensor_add(nf[32:32 + SPC, :], ps_xn, bn)
            nc.vector.tensor_add(nf[64:64 + SPC, :], ps_yn, bp)
            nc.vector.tensor_add(nf[96:96 + SPC, :], ps_yn, bn)
            nfh, nfl = hilo(nf, "nf")

            # assembly: per-slot row DMAs (plain 2D APs)
            for o in range(2):
                dlo = 0 if o == 0 else 32         # lhsT data rows in xyh/xyl
                dro = 32 if o == 0 else 0         # rhs data rows
                rvo = 0 if o == 0 else 64         # row-vector base in nfh/nfl
                cvo = 96 if o == 0 else 32        # col-vector base
                for g in range(2):
                    L = lhsT_t[o][g]
                    R = rhs_t[o][g]
                    for s in range(4):
                        j = g * 4 + s
                        p0 = 32 * s
                        dl = slice(dlo + 4 * j, dlo + 4 * j + 4)
                        dr = slice(dro + 4 * j, dro + 4 * j + 4)
                        rv = slice(rvo + j, rvo + j + 1)
                        cv = slice(cvo + j, cvo + j + 1)
                        nc.sync.dma_start(out=L[p0:p0 + 4, :], in_=xyh[dl])
                        nc.sync.dma_start(out=L[p0 + 4:p0 + 8, :], in_=xyh[dl])
                        nc.sync.dma_start(out=L[p0 + 8:p0 + 12, :], in_=xyl[dl])
                        nc.sync.dma_start(out=L[p0 + 12:p0 + 13, :], in_=nfh[rv])
                        nc.sync.dma_start(out=L[p0 + 13:p0 + 14, :], in_=nfl[rv])
                        nc.sync.dma_start(out=L[p0 + 14:p0 + 16, :], in_=ones2)
                        nc.sync.dma_start(out=R[p0:p0 + 4, :], in_=xyh[dr])
                        nc.sync.dma_start(out=R[p0 + 4:p0 + 8, :], in_=xyl[dr])
                        nc.sync.dma_start(out=R[p0 + 8:p0 + 12, :], in_=xyh[dr])
                        nc.sync.dma_start(out=R[p0 + 12:p0 + 14, :], in_=ones2)
                        nc.sync.dma_start(out=R[p0 + 14:p0 + 15, :], in_=nfh[cv])
                        nc.sync.dma_start(out=R[p0 + 15:p0 + 16, :], in_=nfl[cv])

        # ---- main loop ----
        with tc.tile_pool(name="stage", bufs=4) as stage, \
                tc.tile_pool(name="mm_ps", bufs=2, space="PSUM") as mm_ps:
            for o in range(2):
                for g in range(2):
                    for s in range(4):
                        j = g * 4 + s
                        p0 = 32 * s
                        L = lhsT_t[o][g]
                        R = rhs_t[o][g]
                        for i in range(NB):
                            lhs = L[p0:p0 + 16, i * 128:(i + 1) * 128]
                            ph0 = mm_ps.tile([128, 1024], f32, tag="ph0")
                            ph1 = mm_ps.tile([128, 1024], f32, tag="ph1")
                            for c in range(2):
                                nc.tensor.matmul(
                                    ph0[:, c * 512:(c + 1) * 512], lhs,
                                    R[p0:p0 + 16, c * 512:(c + 1) * 512],
                                    start=True, stop=True,
                                    tile_position=(p0, 0))
                            for c in range(2):
                                nc.tensor.matmul(
                                    ph1[:, c * 512:(c + 1) * 512], lhs,
                                    R[p0:p0 + 16, 1024 + c * 512:1024 + (c + 1) * 512],
                                    start=True, stop=True,
                                    tile_position=(p0, 0))
                            staged = stage.tile([128, 1024], f32, tag="staged")
                            nc.scalar.copy(staged, ph1)
                            mout = stage.tile([128, 1024], f32, tag="mout")
                            nc.vector._custom_dve(
                                max2, out=mout, in0=ph0, in1=staged,
                                s0=NEG_INIT,
                                accum_out=negmax[o][:, j * NB + i:j * NB + i + 1])

        # ---- epilogue ----
        with tc.tile_pool(name="ep", bufs=1) as ep, \
                tc.tile_pool(name="ep_ps", bufs=1, space="PSUM") as ep_ps:
            for o in range(2):
                relu_t = ep.tile([128, 128], f32, tag=f"relu{o}",
                                 name=f"relu{o}")
                nc.scalar.activation(relu_t, negmax[o], Act.Relu,
                                     bias=0.0, scale=-2.0)
                ps = ep_ps.tile([1, 128], f32, tag=f"eps{o}", name=f"eps{o}")
                nc.tensor.matmul(ps, ones_col, relu_t, start=True, stop=True)
                s_sb = ep.tile([1, SPC], f32, tag=f"ssb{o}", name=f"ssb{o}")
                nc.vector.tensor_reduce(
                    s_sb, ps.rearrange("p (s i) -> p s i", s=SPC),
                    Axis.X, Alu.add)
                nc.sync.dma_start(out=sums_out[o:o + 1, :], in_=s_sb)
            cnt_sb = ep.tile([SPC, 1], f32, tag="cnt")
            nc.vector.tensor_reduce(cnt_sb, m_sb, Axis.X, Alu.add)
            nc.sync.dma_start(out=cnt_out, in_=cnt_sb)

    nc.compile()
    return nc


def kernel(recon, target, mask):
    if "nc" not in _CACHE:
        _CACHE["nc"] = _build_bass()
    nc = _CACHE["nc"]
    from concourse.bass_utils import run_bass_kernel_spmd

    recon = np.ascontiguousarray(recon, dtype=np.float32)
    target = np.ascontiguousarray(target, dtype=np.float32)
    maskf = np.ascontiguousarray(mask.astype(np.float32))

    in_maps = []
    for c in range(N_CORES):
        sl = slice(c * SPC, (c + 1) * SPC)
        in_maps.append({
            "recon": np.ascontiguousarray(recon[sl]),
            "target": np.ascontiguousarray(target[sl]),
            "maskf": np.ascontiguousarray(maskf[sl]),
        })

    res = run_bass_kernel_spmd(nc, in_maps, core_ids=list(range(N_CORES)))

    loss_sum = 0.0
    for r in res.results:
        s = r["sums"].astype(np.float64)
        cnt = r["cnt"].astype(np.float64).ravel()
        loss_sum += float(np.sum((s[0] + s[1]) / cnt))
    loss = loss_sum / B
    return np.array(loss, dtype=np.float32)



# revision 7
# speedup vs baseline: 2.2643x; 2.2643x over previous
"""Chamfer distance kernel for Trainium2 (Bass/Tile), 8-core SPMD.

Problem: recon/target [64, 4, 2048] f32, mask [64, 2048] i32 ->
scalar mean chamfer loss (squared distances, masked min both directions).

Strategy (data-parallel over batch, 8 samples/core):
  - Host-side compaction: the mask selects the SAME valid points for both
    recon and target, so dropping masked points is an exact transform.
    Valid points are gathered per sample and padded to NC (multiple of 128,
    1152 for the stock inputs).  All N^2 device work shrinks by (NC/N)^2.
  - Per sample the halved negated pairwise distance matrix
        V[n, m] = x_n . y_m - (xn[n] - BIGr[n])/2 - (yn[m] + BIGc[m])/2
    is produced by ONE K=16 bf16 matmul per tile using an error-free-style
    split (x = xhi + xlo in bf16; dot = xhi.yhi + xhi.ylo + xlo.yhi, the
    dropped xlo.ylo term is ~2^-18 relative); bf16 streams 1 column/cycle
    on the PE (fp32 is 4x slower) and accumulates exactly in fp32 PSUM.
    Row-side bias rows carry +BIG*(1-mask)/2 (padded rows -> V=+BIG/2 ->
    relu(-2*max)=0: no mask multiply needed), column-side bias rows carry
    -BIG*(1-mask)/2 (padded columns excluded from the max).
    max_m V = -d2min/2, recovered exactly by relu(-2*max) in the epilogue.
  - Per 128-row block the PE fills PSUM as two [128, NC/2] tiles;
    ScalarE stages the second half to SBUF (DVE has one PSUM read port);
    one VectorE MAX2_REDUCE custom-DVE op (out = max(in0,in1), accum =
    row max) absorbs both halves in a single pass (DVE cost is max operand
    free-size, so the 2-port fold absorbs the whole block at 0.5
    cycles/element).  Both chamfer directions run as separate matmul
    orientations.
  - Four samples pack per 128-partition operand tensor at 32-partition
    slots (matmul lhsT base-partition constraint) with explicit
    tile_position; slot rows as in the bf16 split layout (ones rows come
    from a tile-wide memset instead of DMAs).
  - Epilogue: relu(-2*max) on ScalarE, partition sum via ones-matmul,
    block sums via a 3D-AP reduce.  Output per core: sums [2, 8]; the
    masked means and batch mean happen on host (counts are host-known).
"""

import sys

import numpy as np

for _p in ("/opt/trn_rl_repo",):
    if _p not in sys.path:
        sys.path.append(_p)

B, F, N = 64, 4, 2048
N_CORES = 8
SPC = B // N_CORES  # samples per core
BIGV = 1.0e30
NEG_INIT = -3.0e38

_CACHE = {}



def _register_max2_reduce():
    """Author + register a custom DVE op: out = max(in0, in1),
    accum_out = max-reduce(out) seeded from s0.  Absorbs two PSUM tiles per
    pass (one read port each) with the row-max fused — DVE cost is the max
    operand free-size, so this absorbs a whole block at 0.5 cycles/elem."""
    from concourse import dve_ops
    from concourse.dve_spec import Spec, Src0, Src1, C0, maxx, lower, _has_src1
    from concourse.dve_uop import DveOpSpec
    import numpy as np

    NAME = "MAX2_REDUCE_ANT"
    for op in dve_ops.OPS:
        if op.name == NAME:
            return op

    def _ref_max2(in0, in1, c0, c1, c2):
        b = np.maximum(in0.astype(np.float32), in1.astype(np.float32))
        a = np.maximum(b.reshape(b.shape[0], -1).max(axis=-1, keepdims=True), c0)
        return b, a

    spec = Spec(body=maxx(Src0, Src1), accum=maxx, accum_init=C0,
                reference=_ref_max2)
    row = dve_ops._CUSTOM_DVE_ROW_BASE + len(dve_ops.OPS)
    shas = {}
    for ver in ("v3", "v4"):
        s = DveOpSpec(name=NAME, opcode=row, uops=lower(spec, ver=ver),
                      rd1_en=_has_src1(spec))
        shas[ver] = s.sha(ver)
    op = dve_ops.DveOp(NAME, spec, subdim=False, uops_sha=shas)
    dve_ops.OPS.append(op)
    dve_ops._SUB_OPCODE_FOR_NAME[NAME] = row
    dve_ops.CUSTOM_DVE_SPECS[NAME] = spec
    return op


def _build_bass(NC):
    from contextlib import ExitStack

    import concourse.mybir as mybir
    import concourse.tile as tile
    from concourse import bacc

    NB = NC // 128  # 128-row blocks per sample
    HALF = NC // 2

    f32 = mybir.dt.float32
    bf16 = mybir.dt.bfloat16
    Alu = mybir.AluOpType
    Act = mybir.ActivationFunctionType
    Axis = mybir.AxisListType

    max2 = _register_max2_reduce()

    nc = bacc.Bacc("TRN2", target_bir_lowering=False, debug=False,
                   num_devices=N_CORES)

    recon = nc.dram_tensor("recon", (SPC, F, NC), f32,
                           kind="ExternalInput").ap()
    target = nc.dram_tensor("target", (SPC, F, NC), f32,
                            kind="ExternalInput").ap()
    bp_d = nc.dram_tensor("bp", (SPC, NC), f32, kind="ExternalInput").ap()
    bn_d = nc.dram_tensor("bn", (SPC, NC), f32, kind="ExternalInput").ap()
    sums_out = nc.dram_tensor("sums", (2, SPC), f32, kind="ExternalOutput").ap()

    with tile.TileContext(nc) as tc, ExitStack() as ctx:
        # ---- persistent pools ----
        consts = ctx.enter_context(tc.tile_pool(name="consts", bufs=1))
        opnds = ctx.enter_context(tc.tile_pool(name="opnds", bufs=1))
        accum = ctx.enter_context(tc.tile_pool(name="accum", bufs=1))

        ones_col = consts.tile([128, 1], f32)
        nc.gpsimd.memset(ones_col, 1.0)
        # negE [64, 8]: -0.5 on the (4-row) block diagonal, at partition 0
        # (for x) and partition 32 (for y) -> norm matmuls give -|p|^2/2
        negE = consts.tile([2 * SPC * F, SPC], f32, name="negE")
        nc.gpsimd.memset(negE, -0.5)
        for base in (0, 32):
            nc.gpsimd.affine_select(out=negE[base:base + 32, :],
                                    in_=negE[base:base + 32, :],
                                    compare_op=Alu.is_ge, fill=0.0,
                                    base=0, pattern=[[-F, SPC]],
                                    channel_multiplier=1)
            nc.gpsimd.affine_select(out=negE[base:base + 32, :],
                                    in_=negE[base:base + 32, :],
                                    compare_op=Alu.is_ge, fill=0.0,
                                    base=F - 1, pattern=[[F, SPC]],
                                    channel_multiplier=-1)

        # operand tensors (bf16): [orientation][group]; sample slot s lives
        # at partitions [32s, 32s+16):
        #   lhsT rows: 0-3 xhi | 4-7 xhi | 8-11 xlo | 12 rvh | 13 rvl | 14-15 1
        #   rhs  rows: 0-3 yhi | 4-7 ylo | 8-11 yhi | 12-13 1 | 14 cvh | 15 cvl
        # memset(1.0) provides the ones rows; DMAs overwrite the rest.
        lhsT_t = [[opnds.tile([128, NC], bf16, tag=f"L{o}{g}", name=f"L{o}{g}")
                   for g in range(2)] for o in range(2)]
        rhs_t = [[opnds.tile([128, NC], bf16, tag=f"R{o}{g}", name=f"R{o}{g}")
                  for g in range(2)] for o in range(2)]
        for o in range(2):
            for g in range(2):
                nc.gpsimd.memset(lhsT_t[o][g], 1.0)
                nc.gpsimd.memset(rhs_t[o][g], 1.0)
        negmax = [accum.tile([128, SPC * NB], f32, tag=f"nm{o}", name=f"nm{o}")
                  for o in range(2)]

        # ---- prep: hi/lo splits, norms, masked bias vectors, assembly ----
        with tc.tile_pool(name="prep_a", bufs=1) as prep_a, \
                tc.tile_pool(name="prep_b", bufs=1) as prep_b, \
                tc.tile_pool(name="prep_ps", bufs=1, space="PSUM") as prep_ps:
            # x at partitions 0-31, y at partitions 32-63
            xy = prep_a.tile([2 * SPC * F, NC], f32, tag="xy")
            nc.sync.dma_start(out=xy[:SPC * F, :],
                              in_=recon.rearrange("b f n -> (b f) n"))
            nc.sync.dma_start(out=xy[SPC * F:, :],
                              in_=target.rearrange("b f n -> (b f) n"))
            bp = prep_a.tile([SPC, NC], f32, tag="bp_s", name="bp_s")
            bn = prep_a.tile([SPC, NC], f32, tag="bn_s", name="bn_s")
            nc.sync.dma_start(out=bp, in_=bp_d)
            nc.sync.dma_start(out=bn, in_=bn_d)

            def hilo(src, tag):
                """bf16 hi/lo split: hi = bf16(src), lo = bf16(src - hi).
                The f32 diff scratch shares one slot across all splits."""
                p = src.shape[0]
                hi = prep_b.tile([p, NC], bf16, tag=f"{tag}_h", name=f"{tag}_h")
                df = prep_a.tile([128, NC], f32, tag="hilo_d", name=f"{tag}_d")
                lo = prep_b.tile([p, NC], bf16, tag=f"{tag}_l", name=f"{tag}_l")
                nc.scalar.copy(hi, src)
                nc.vector.tensor_sub(df[:p], src, hi)
                nc.scalar.copy(lo, df[:p])
                return hi, lo

            xyh, xyl = hilo(xy, "xy")

            sq = prep_a.tile([2 * SPC * F, NC], f32, tag="sq")
            nc.scalar.square(sq[:SPC * F, :], xy[:SPC * F, :])
            nc.scalar.square(sq[SPC * F:, :], xy[SPC * F:, :])

            # -xn/2, -yn/2 via block-diagonal -(1/2) ones matmuls (K=32, M=8)
            ps_xn = prep_ps.tile([SPC, NC], f32, tag="psxn")
            ps_yn = prep_ps.tile([SPC, NC], f32, tag="psyn")
            chunks = [(c * 512, min(512, NC - c * 512))
                      for c in range((NC + 511) // 512)]
            for c0, w in chunks:
                sl = slice(c0, c0 + w)
                nc.tensor.matmul(ps_xn[:, sl], negE[0:32, :], sq[0:32, sl],
                                 start=True, stop=True, tile_position=(0, 0))
                nc.tensor.matmul(ps_yn[:, sl], negE[32:64, :], sq[32:64, sl],
                                 start=True, stop=True, tile_position=(32, 0))

            # masked bias vectors at 32-aligned partition bases (engine APs
            # must start at a 32-aligned partition): rows 0-7 rv_x | 32-39
            # cv_x | 64-71 rv_y | 96-103 cv_y
            # (rv = -n/2 + BIG(1-m)/2, cv = -n/2 - BIG(1-m)/2)
            nf = prep_a.tile([128, NC], f32, tag="nf")
            nc.vector.tensor_add(nf[0:SPC, :], ps_xn, bp)
            nc.vector.tensor_add(nf[32:32 + SPC, :], ps_xn, bn)
            nc.vector.tensor_add(nf[64:64 + SPC, :], ps_yn, bp)
            nc.vector.tensor_add(nf[96:96 + SPC, :], ps_yn, bn)
            nfh, nfl = hilo(nf, "nf")

            # assembly: per-slot row DMAs (ones rows already memset)
            for o in range(2):
                dlo = 0 if o == 0 else 32         # lhsT data rows in xyh/xyl
                dro = 32 if o == 0 else 0         # rhs data rows
                rvo = 0 if o == 0 else 64         # row-vector base in nfh/nfl
                cvo = 96 if o == 0 else 32        # col-vector base
                for g in range(2):
                    L = lhsT_t[o][g]
                    R = rhs_t[o][g]
                    for s in range(4):
                        j = g * 4 + s
                        p0 = 32 * s
                        dl = slice(dlo + 4 * j, dlo + 4 * j + 4)
                        dr = slice(dro + 4 * j, dro + 4 * j + 4)
                        rv = slice(rvo + j, rvo + j + 1)
                        cv = slice(cvo + j, cvo + j + 1)
                        nc.sync.dma_start(out=L[p0:p0 + 4, :], in_=xyh[dl])
                        nc.sync.dma_start(out=L[p0 + 4:p0 + 8, :], in_=xyh[dl])
                        nc.sync.dma_start(out=L[p0 + 8:p0 + 12, :], in_=xyl[dl])
                        nc.sync.dma_start(out=L[p0 + 12:p0 + 13, :],
                                          in_=nfh[rv])
                        nc.sync.dma_start(out=L[p0 + 13:p0 + 14, :],
                                          in_=nfl[rv])
                        nc.sync.dma_start(out=R[p0:p0 + 4, :], in_=xyh[dr])
                        nc.sync.dma_start(out=R[p0 + 4:p0 + 8, :], in_=xyl[dr])
                        nc.sync.dma_start(out=R[p0 + 8:p0 + 12, :],
                                          in_=xyh[dr])
                        nc.sync.dma_start(out=R[p0 + 14:p0 + 15, :],
                                          in_=nfh[cv])
                        nc.sync.dma_start(out=R[p0 + 15:p0 + 16, :],
                                          in_=nfl[cv])

        # ---- main loop ----
        with tc.tile_pool(name="stage", bufs=4) as stage, \
                tc.tile_pool(name="mm_ps", bufs=2, space="PSUM") as mm_ps:
            for o in range(2):
                for g in range(2):
                    for s in range(4):
                        j = g * 4 + s
                        p0 = 32 * s
                        L = lhsT_t[o][g]
                        R = rhs_t[o][g]
                        for i in range(NB):
                            lhs = L[p0:p0 + 16, i * 128:(i + 1) * 128]
                            ph0 = mm_ps.tile([128, HALF], f32, tag="ph0")
                            ph1 = mm_ps.tile([128, HALF], f32, tag="ph1")
                            for base, ph in ((0, ph0), (HALF, ph1)):
                                for c0, w in ((0, 512), (512, HALF - 512)):
                                    nc.tensor.matmul(
                                        ph[:, c0:c0 + w], lhs,
                                        R[p0:p0 + 16,
                                          base + c0:base + c0 + w],
                                        start=True, stop=True,
                                        tile_position=(p0, 0))
                            staged = stage.tile([128, HALF], f32,
                                                tag="staged")
                            nc.scalar.copy(staged, ph1)
                            junk = stage.tile([128, HALF], f32, tag="junk")
                            col = j * NB + i
                            nc.vector._custom_dve(
                                max2, out=junk, in0=ph0, in1=staged,
                                s0=NEG_INIT,
                                accum_out=negmax[o][:, col:col + 1])

        # ---- epilogue ----
        with tc.tile_pool(name="ep", bufs=1) as ep, \
                tc.tile_pool(name="ep_ps", bufs=1, space="PSUM") as ep_ps:
            for o in range(2):
                relu_t = ep.tile([128, SPC * NB], f32, tag=f"relu{o}",
                                 name=f"relu{o}")
                nc.scalar.activation(relu_t, negmax[o], Act.Relu,
                                     bias=0.0, scale=-2.0)
                ps = ep_ps.tile([1, SPC * NB], f32, tag=f"eps{o}",
                                name=f"eps{o}")
                nc.tensor.matmul(ps, ones_col, relu_t, start=True, stop=True)
                s_sb = ep.tile([1, SPC], f32, tag=f"ssb{o}", name=f"ssb{o}")
                nc.vector.tensor_reduce(
                    s_sb, ps.rearrange("p (s i) -> p s i", s=SPC),
                    Axis.X, Alu.add)
                nc.sync.dma_start(out=sums_out[o:o + 1, :], in_=s_sb)

    nc.compile()
    return nc


def _compact(recon, target, mask):
    """Gather valid points per sample, pad to a common NC (multiple of 128).
    Exact: the same mask gates rows (via sum mask) and columns (via BIG) in
    the reference, so dropped points cannot affect any min or sum."""
    recon = np.ascontiguousarray(recon, dtype=np.float32)
    target = np.ascontiguousarray(target, dtype=np.float32)
    mask = np.asarray(mask)
    cnt = mask.astype(np.int64).sum(axis=1)
    NC = max(256, int(-(-int(cnt.max()) // 128) * 128))
    rc = np.zeros((B, F, NC), dtype=np.float32)
    tc_ = np.zeros((B, F, NC), dtype=np.float32)
    mc = np.zeros((B, NC), dtype=np.float32)
    for s in range(B):
        idx = np.flatnonzero(mask[s])
        c = idx.size
        rc[s, :, :c] = recon[s][:, idx]
        tc_[s, :, :c] = target[s][:, idx]
        mc[s, :c] = 1.0
    return rc, tc_, mc, cnt.astype(np.float64), NC


def make_in_maps(inputs):
    """Per-core device input dicts (compacted); also returns counts, NC."""
    rc, tc_, mc, cnt, NC = _compact(inputs["recon"], inputs["target"],
                                    inputs["mask"])
    bp_full = (BIGV / 2) * (1.0 - mc)
    bn_full = -bp_full
    in_maps = []
    for c in range(N_CORES):
        sl = slice(c * SPC, (c + 1) * SPC)
        in_maps.append({
            "recon": np.ascontiguousarray(rc[sl]),
            "target": np.ascontiguousarray(tc_[sl]),
            "bp": np.ascontiguousarray(bp_full[sl], dtype=np.float32),
            "bn": np.ascontiguousarray(bn_full[sl], dtype=np.float32),
        })
    return in_maps, cnt, NC


def kernel(recon, target, mask):
    in_maps, cnt, NC = make_in_maps(
        {"recon": recon, "target": target, "mask": mask})
    if _CACHE.get("NC") != NC:
        _CACHE["nc"] = _build_bass(NC)
        _CACHE["NC"] = NC
    nc = _CACHE["nc"]
    from concourse.bass_utils import run_bass_kernel_spmd

    res = run_bass_kernel_spmd(nc, in_maps, core_ids=list(range(N_CORES)))

    loss_sum = 0.0
    for c, r in enumerate(res.results):
        s = r["sums"].astype(np.float64)
        loss_sum += float(np.sum((s[0] + s[1]) / cnt[c * SPC:(c + 1) * SPC]))
    loss = loss_sum / B
    return np.array(loss, dtype=np.float32)


# revision 15
# speedup vs baseline: 2.5556x; 1.1286x over previous
"""Chamfer distance kernel for Trainium2 (Bass/Tile), 8-core SPMD.

Problem: recon/target [64, 4, 2048] f32, mask [64, 2048] i32 ->
scalar mean chamfer loss (squared distances, masked min both directions).

Strategy (data-parallel over batch, 8 samples/core):
  - Host-side compaction: the mask selects the SAME valid points for both
    recon and target, so dropping masked points is an exact transform.
    Valid points are gathered per sample; samples are sorted by count and
    dealt round-robin to cores so that slot j holds similar-sized samples
    on every core.  Slot j's loops are sized to cap_j = the max count in
    that slot (bucketing) — all N^2 device work shrinks by ~(cap/N)^2.
  - Per sample the halved negated pairwise distance matrix
        V[n, m] = x_n . y_m - (xn[n] - BIGr[n])/2 - (yn[m] + BIGc[m])/2
    is produced by ONE K=16 bf16 matmul per tile using an error-free-style
    split (x = xhi + xlo in bf16; dot = xhi.yhi + xhi.ylo + xlo.yhi, the
    dropped xlo.ylo term is ~2^-18 relative); bf16 streams 1 column/cycle
    on the PE (fp32 is 4x slower) and accumulates exactly in fp32 PSUM.
    Row-side bias rows carry +BIG*(1-mask)/2 (padded rows -> V=+BIG/2 ->
    relu(-2*max)=0: no mask multiply needed), column-side bias rows carry
    -BIG*(1-mask)/2 (padded columns excluded from the max).
    max_m V = -d2min/2, recovered exactly by relu(-2*max) in the epilogue.
  - Per row-block the PE fills PSUM as two [rows, cap/2] tiles; ScalarE
    stages the second half to SBUF (DVE has one PSUM read port); one
    VectorE MAX2_REDUCE custom-DVE op (out = max(in0,in1), accum = row
    max) absorbs both halves in a single pass (DVE cost is max operand
    free-size, so the 2-port fold absorbs the whole block at 0.5
    cycles/element).  Both chamfer directions run as separate matmul
    orientations.
  - Four samples pack per 128-partition operand tensor at 32-partition
    slots (matmul lhsT base-partition constraint) with explicit
    tile_position; ones rows come from a tile-wide memset instead of DMAs.
  - negmax accumulators are memset to +1.0 so row-blocks shorter than 128
    rows / slots with fewer blocks contribute relu(-2*1) = 0.
  - Epilogue: relu(-2*max) on ScalarE, partition sum via ones-matmul,
    block sums via a 3D-AP reduce.  Output per core: sums [2, 8]; the
    masked means and batch mean happen on host (counts are host-known).
"""

import sys

import numpy as np

for _p in ("/opt/trn_rl_repo",):
    if _p not in sys.path:
        sys.path.append(_p)

B, F, N = 64, 4, 2048
N_CORES = 8
SPC = B // N_CORES  # samples per core
BIGV = 1.0e30
NEG_INIT = -3.0e38

_CACHE = {}


def _register_max2_reduce():
    """Author + register a custom DVE op: out = max(in0, in1),
    accum_out = max-reduce(out) seeded from s0.  Absorbs two tiles per pass
    (one read port each) with the row-max fused — the core absorption
    primitive of this kernel."""
    from concourse import dve_ops
    from concourse.dve_spec import Spec, Src0, Src1, C0, maxx, lower, _has_src1
    from concourse.dve_uop import DveOpSpec

    NAME = "MAX2_REDUCE_ANT"
    for op in dve_ops.OPS:
        if op.name == NAME:
            return op

    def _ref_max2(in0, in1, c0, c1, c2):
        b = np.maximum(in0.astype(np.float32), in1.astype(np.float32))
        a = np.maximum(b.reshape(b.shape[0], -1).max(axis=-1, keepdims=True), c0)
        return b, a

    spec = Spec(body=maxx(Src0, Src1), accum=maxx, accum_init=C0,
                reference=_ref_max2)
    row = dve_ops._CUSTOM_DVE_ROW_BASE + len(dve_ops.OPS)
    shas = {}
    for ver in ("v3", "v4"):
        s = DveOpSpec(name=NAME, opcode=row, uops=lower(spec, ver=ver),
                      rd1_en=_has_src1(spec))
        shas[ver] = s.sha(ver)
    op = dve_ops.DveOp(NAME, spec, subdim=False, uops_sha=shas)
    dve_ops.OPS.append(op)
    dve_ops._SUB_OPCODE_FOR_NAME[NAME] = row
    dve_ops.CUSTOM_DVE_SPECS[NAME] = spec
    return op


def _ru(x, m):
    return -(-int(x) // m) * m


def _build_bass(caps):
    from contextlib import ExitStack

    import concourse.mybir as mybir
    import concourse.tile as tile
    from concourse import bacc

    CW = [_ru(c, 64) for c in caps]      # per-slot column width
    NC = max(CW)                         # operand/prep tensor width
    RB = [-(-c // 128) for c in caps]    # row blocks per slot
    RBMAX = max(RB)

    f32 = mybir.dt.float32
    bf16 = mybir.dt.bfloat16
    Alu = mybir.AluOpType
    Act = mybir.ActivationFunctionType
    Axis = mybir.AxisListType

    max2 = _register_max2_reduce()

    nc = bacc.Bacc("TRN2", target_bir_lowering=False, debug=False,
                   num_devices=N_CORES)

    recon = nc.dram_tensor("recon", (SPC, F, NC), f32,
                           kind="ExternalInput").ap()
    target = nc.dram_tensor("target", (SPC, F, NC), f32,
                            kind="ExternalInput").ap()
    bp_d = nc.dram_tensor("bp", (SPC, NC), f32, kind="ExternalInput").ap()
    bn_d = nc.dram_tensor("bn", (SPC, NC), f32, kind="ExternalInput").ap()
    sums_out = nc.dram_tensor("sums", (2, SPC), f32, kind="ExternalOutput").ap()

    with tile.TileContext(nc) as tc, ExitStack() as ctx:
        # ---- persistent pools ----
        consts = ctx.enter_context(tc.tile_pool(name="consts", bufs=1))
        opnds = ctx.enter_context(tc.tile_pool(name="opnds", bufs=1))
        accum = ctx.enter_context(tc.tile_pool(name="accum", bufs=1))

        ones_col = consts.tile([128, 1], f32)
        nc.gpsimd.memset(ones_col, 1.0)
        # negE [64, 8]: -0.5 on the (4-row) block diagonal, at partition 0
        # (for x) and partition 32 (for y) -> norm matmuls give -|p|^2/2
        negE = consts.tile([2 * SPC * F, SPC], f32, name="negE")
        nc.gpsimd.memset(negE, -0.5)
        for base in (0, 32):
            nc.gpsimd.affine_select(out=negE[base:base + 32, :],
                                    in_=negE[base:base + 32, :],
                                    compare_op=Alu.is_ge, fill=0.0,
                                    base=0, pattern=[[-F, SPC]],
                                    channel_multiplier=1)
            nc.gpsimd.affine_select(out=negE[base:base + 32, :],
                                    in_=negE[base:base + 32, :],
                                    compare_op=Alu.is_ge, fill=0.0,
                                    base=F - 1, pattern=[[F, SPC]],
                                    channel_multiplier=-1)

        # operand tensors (bf16): [orientation][group]; sample slot s lives
        # at partitions [32s, 32s+16):
        #   lhsT rows: 0-3 xhi | 4-7 xhi | 8-11 xlo | 12 rvh | 13 rvl | 14-15 1
        #   rhs  rows: 0-3 yhi | 4-7 ylo | 8-11 yhi | 12-13 1 | 14 cvh | 15 cvl
        # memset(1.0) provides the ones rows; DMAs overwrite the rest.
        lhsT_t = [[opnds.tile([128, NC], bf16, tag=f"L{o}{g}", name=f"L{o}{g}")
                   for g in range(2)] for o in range(2)]
        rhs_t = [[opnds.tile([128, NC], bf16, tag=f"R{o}{g}", name=f"R{o}{g}")
                  for g in range(2)] for o in range(2)]
        for o in range(2):
            for g in range(2):
                nc.gpsimd.memset(lhsT_t[o][g], 1.0)
                nc.gpsimd.memset(rhs_t[o][g], 1.0)
        # +1.0 init: untouched (row, col) entries yield relu(-2*1) = 0
        negmax = [accum.tile([128, SPC * RBMAX], f32, tag=f"nm{o}",
                             name=f"nm{o}") for o in range(2)]
        nc.gpsimd.memset(negmax[0], 1.0)
        nc.gpsimd.memset(negmax[1], 1.0)

        # ---- prep: hi/lo splits, norms, masked bias vectors, assembly ----
        with tc.tile_pool(name="prep_a", bufs=1) as prep_a, \
                tc.tile_pool(name="prep_b", bufs=1) as prep_b, \
                tc.tile_pool(name="prep_ps", bufs=1, space="PSUM") as prep_ps:
            # x at partitions 0-31, y at partitions 32-63
            xy = prep_a.tile([2 * SPC * F, NC], f32, tag="xy")
            nc.sync.dma_start(out=xy[:SPC * F, :],
                              in_=recon.rearrange("b f n -> (b f) n"))
            nc.sync.dma_start(out=xy[SPC * F:, :],
                              in_=target.rearrange("b f n -> (b f) n"))
            bp = prep_a.tile([SPC, NC], f32, tag="bp_s", name="bp_s")
            bn = prep_a.tile([SPC, NC], f32, tag="bn_s", name="bn_s")
            nc.sync.dma_start(out=bp, in_=bp_d)
            nc.sync.dma_start(out=bn, in_=bn_d)

            def hilo(src, tag):
                """bf16 hi/lo split: hi = bf16(src), lo = bf16(src - hi).
                The f32 diff scratch shares one slot across all splits."""
                p = src.shape[0]
                hi = prep_b.tile([p, NC], bf16, tag=f"{tag}_h", name=f"{tag}_h")
                df = prep_a.tile([128, NC], f32, tag="hilo_d", name=f"{tag}_d")
                lo = prep_b.tile([p, NC], bf16, tag=f"{tag}_l", name=f"{tag}_l")
                nc.scalar.copy(hi, src)
                nc.vector.tensor_sub(df[:p], src, hi)
                nc.scalar.copy(lo, df[:p])
                return hi, lo

            xyh, xyl = hilo(xy, "xy")

            sq = prep_a.tile([2 * SPC * F, NC], f32, tag="sq")
            nc.scalar.square(sq[:SPC * F, :], xy[:SPC * F, :])
            nc.scalar.square(sq[SPC * F:, :], xy[SPC * F:, :])

            # -xn/2, -yn/2 via block-diagonal -(1/2) ones matmuls (K=32, M=8)
            ps_xn = prep_ps.tile([SPC, NC], f32, tag="psxn")
            ps_yn = prep_ps.tile([SPC, NC], f32, tag="psyn")
            chunks = [(c * 512, min(512, NC - c * 512))
                      for c in range((NC + 511) // 512)]
            for c0, w in chunks:
                sl = slice(c0, c0 + w)
                nc.tensor.matmul(ps_xn[:, sl], negE[0:32, :], sq[0:32, sl],
                                 start=True, stop=True, tile_position=(0, 0))
                nc.tensor.matmul(ps_yn[:, sl], negE[32:64, :], sq[32:64, sl],
                                 start=True, stop=True, tile_position=(32, 0))

            # masked bias vectors at 32-aligned partition bases (engine APs
            # must start at a 32-aligned partition): rows 0-7 rv_x | 32-39
            # cv_x | 64-71 rv_y | 96-103 cv_y
            # (rv = -n/2 + BIG(1-m)/2, cv = -n/2 - BIG(1-m)/2)
            nf = prep_a.tile([128, NC], f32, tag="nf")
            nc.vector.tensor_add(nf[0:SPC, :], ps_xn, bp)
            nc.vector.tensor_add(nf[32:32 + SPC, :], ps_xn, bn)
            nc.vector.tensor_add(nf[64:64 + SPC, :], ps_yn, bp)
            nc.vector.tensor_add(nf[96:96 + SPC, :], ps_yn, bn)
            nfh, nfl = hilo(nf, "nf")

            # assembly: per-slot row DMAs (ones rows already memset).  The
            # first group's rhs goes via the idle Act queue so the first
            # matmuls start sooner (SP sequencer is the prologue bottleneck).
            for o in range(2):
                dlo = 0 if o == 0 else 32         # lhsT data rows in xyh/xyl
                dro = 32 if o == 0 else 0         # rhs data rows
                rvo = 0 if o == 0 else 64         # row-vector base in nfh/nfl
                cvo = 96 if o == 0 else 32        # col-vector base
                for g in range(2):
                    L = lhsT_t[o][g]
                    R = rhs_t[o][g]
                    rq = nc.scalar if (o == 0 and g == 0) else nc.sync
                    for s in range(4):
                        j = g * 4 + s
                        p0 = 32 * s
                        dl = slice(dlo + 4 * j, dlo + 4 * j + 4)
                        dr = slice(dro + 4 * j, dro + 4 * j + 4)
                        rv = slice(rvo + j, rvo + j + 1)
                        cv = slice(cvo + j, cvo + j + 1)
                        nc.sync.dma_start(out=L[p0:p0 + 4, :], in_=xyh[dl])
                        nc.sync.dma_start(out=L[p0 + 4:p0 + 8, :], in_=xyh[dl])
                        nc.sync.dma_start(out=L[p0 + 8:p0 + 12, :], in_=xyl[dl])
                        nc.sync.dma_start(out=L[p0 + 12:p0 + 13, :],
                                          in_=nfh[rv])
                        nc.sync.dma_start(out=L[p0 + 13:p0 + 14, :],
                                          in_=nfl[rv])
                        rq.dma_start(out=R[p0:p0 + 4, :], in_=xyh[dr])
                        rq.dma_start(out=R[p0 + 4:p0 + 8, :], in_=xyl[dr])
                        rq.dma_start(out=R[p0 + 8:p0 + 12, :], in_=xyh[dr])
                        rq.dma_start(out=R[p0 + 14:p0 + 15, :], in_=nfh[cv])
                        rq.dma_start(out=R[p0 + 15:p0 + 16, :], in_=nfl[cv])

        # ---- main loop ----
        with tc.tile_pool(name="stage", bufs=4) as stage, \
                tc.tile_pool(name="mm_ps", bufs=2, space="PSUM") as mm_ps:
            for o in range(2):
                for g in range(2):
                    for s in range(4):
                        j = g * 4 + s
                        p0 = 32 * s
                        L = lhsT_t[o][g]
                        R = rhs_t[o][g]
                        half = CW[j] // 2
                        ccs = [(0, min(512, half))]
                        if half > 512:
                            ccs.append((512, half - 512))
                        for i in range(RB[j]):
                            r0 = i * 128
                            rows = min(128, caps[j] - r0)
                            lhs = L[p0:p0 + 16, r0:r0 + rows]
                            ph0 = mm_ps.tile([128, 576], f32, tag="ph0")
                            ph1 = mm_ps.tile([128, 576], f32, tag="ph1")
                            for base, ph in ((0, ph0), (half, ph1)):
                                for c0, w in ccs:
                                    nc.tensor.matmul(
                                        ph[:rows, c0:c0 + w], lhs,
                                        R[p0:p0 + 16,
                                          base + c0:base + c0 + w],
                                        start=True, stop=True,
                                        tile_position=(p0, 0))
                            staged = stage.tile([128, 576], f32, tag="staged")
                            nc.scalar.copy(staged[:rows, :half],
                                           ph1[:rows, :half])
                            junk = stage.tile([128, 576], f32, tag="junk")
                            col = j * RBMAX + i
                            nc.vector._custom_dve(
                                max2, out=junk[:rows, :half],
                                in0=ph0[:rows, :half],
                                in1=staged[:rows, :half],
                                s0=NEG_INIT,
                                accum_out=negmax[o][0:rows, col:col + 1])

        # ---- epilogue ----
        with tc.tile_pool(name="ep", bufs=1) as ep, \
                tc.tile_pool(name="ep_ps", bufs=1, space="PSUM") as ep_ps:
            for o in range(2):
                relu_t = ep.tile([128, SPC * RBMAX], f32, tag=f"relu{o}",
                                 name=f"relu{o}")
                nc.scalar.activation(relu_t, negmax[o], Act.Relu,
                                     bias=0.0, scale=-2.0)
                ps = ep_ps.tile([1, SPC * RBMAX], f32, tag=f"eps{o}",
                                name=f"eps{o}")
                nc.tensor.matmul(ps, ones_col, relu_t, start=True, stop=True)
                s_sb = ep.tile([1, SPC], f32, tag=f"ssb{o}", name=f"ssb{o}")
                nc.vector.tensor_reduce(
                    s_sb, ps.rearrange("p (s i) -> p s i", s=SPC),
                    Axis.X, Alu.add)
                nc.sync.dma_start(out=sums_out[o:o + 1, :], in_=s_sb)

    nc.compile()
    return nc


def _compact(recon, target, mask):
    """Gather valid-point indices per sample.  Exact: the same mask gates
    rows (via sum mask) and columns (via BIG) in the reference, so dropped
    points cannot affect any min or sum."""
    recon = np.ascontiguousarray(recon, dtype=np.float32)
    target = np.ascontiguousarray(target, dtype=np.float32)
    mask = np.asarray(mask)
    cnt = mask.astype(np.int64).sum(axis=1)
    idxs = [np.flatnonzero(mask[s]) for s in range(B)]
    return recon, target, idxs, cnt


def make_in_maps(inputs):
    """Per-core device input dicts (compacted + bucketed).  Samples are
    sorted by valid count and dealt round-robin: core c, slot j holds the
    sample of rank 8j + c, so slot capacities are uniform across cores.
    Returns (in_maps, per-core counts [N_CORES, SPC], slot capacities)."""
    recon, target, idxs, cnt = _compact(inputs["recon"], inputs["target"],
                                        inputs["mask"])
    order = np.argsort(-cnt, kind="stable")
    caps = tuple(int(cnt[order[N_CORES * j]]) for j in range(SPC))
    NC = max(_ru(c, 64) for c in caps)
    in_maps = []
    counts = np.zeros((N_CORES, SPC), dtype=np.float64)
    for c in range(N_CORES):
        rc = np.zeros((SPC, F, NC), dtype=np.float32)
        tc_ = np.zeros((SPC, F, NC), dtype=np.float32)
        bp = np.full((SPC, NC), BIGV / 2, dtype=np.float32)
        for j in range(SPC):
            s = order[N_CORES * j + c]
            idx = idxs[s]
            k = idx.size
            rc[j, :, :k] = recon[s][:, idx]
            tc_[j, :, :k] = target[s][:, idx]
            bp[j, :k] = 0.0
            counts[c, j] = k
        in_maps.append({
            "recon": rc,
            "target": tc_,
            "bp": bp,
            "bn": np.ascontiguousarray(-bp),
        })
    return in_maps, counts, caps


def kernel(recon, target, mask):
    in_maps, counts, caps = make_in_maps(
        {"recon": recon, "target": target, "mask": mask})
    if _CACHE.get("caps") != caps:
        _CACHE["nc"] = _build_bass(caps)
        _CACHE["caps"] = caps
    nc = _CACHE["nc"]
    from concourse.bass_utils import run_bass_kernel_spmd

    res = run_bass_kernel_spmd(nc, in_maps, core_ids=list(range(N_CORES)))

    loss_sum = 0.0
    for c, r in enumerate(res.results):
        s = r["sums"].astype(np.float64)
        loss_sum += float(np.sum((s[0] + s[1]) / counts[c]))
    loss = loss_sum / B
    return np.array(loss, dtype=np.float32)
